# revision 9
# baseline (speedup 1.0000x reference)
"""MoE router + dispatch kernel for Trainium2 (Bass/Tile), 8-core data parallel.

Computes, for x = inputs.reshape(B*S, D):
    logits = x @ w                      # [N, E]
    probs  = softmax(logits, axis=-1)
    gate, idx = top_k(probs, k=2)       # [N, 2]
    out[2t+k] = x[t] * gate[t, k]       # [2N, D]
returns (out, idx.reshape(-1)) matching the jax reference.

Sharding: data-parallel on the token dim across 8 NeuronCores; w replicated.
"""

from contextlib import ExitStack

import numpy as np

import concourse.bacc as bacc
import concourse.bass as bass
import concourse.tile as tile
from concourse import mybir
from concourse.bass_utils import run_bass_kernel_spmd
from concourse.masks import make_identity

N_CORES = 8
B, S, D, E, TOPK = 8, 4096, 1024, 64, 2
N_TOK = B * S                  # 32768 tokens total
TOK_CORE = N_TOK // N_CORES    # 4096 tokens per core
P = 128                        # SBUF partitions
NT = TOK_CORE // P             # 32 token tiles per core
DC = D // P                    # 8 contraction chunks

F32 = mybir.dt.float32


def _moe_tile_kernel(ctx: ExitStack, tc: tile.TileContext, x_ap, w_ap, y_ap, idx_ap, nt=NT, reps=1):
    nc = tc.nc

    const = ctx.enter_context(tc.tile_pool(name="const", bufs=1))
    xpool = ctx.enter_context(tc.tile_pool(name="xp", bufs=8))
    xtpool = ctx.enter_context(tc.tile_pool(name="xtp", bufs=3))
    ypool = ctx.enter_context(tc.tile_pool(name="yp", bufs=4))
    small = ctx.enter_context(tc.tile_pool(name="small", bufs=8))
    pst = ctx.enter_context(tc.tile_pool(name="pst", bufs=4, space="PSUM"))
    psl = ctx.enter_context(tc.tile_pool(name="psl", bufs=3, space="PSUM"))

    ident = const.tile([P, P], F32)
    make_identity(nc, ident[:])

    # w pre-chunked on host: w_ap[p, c, e] = w[c*128 + p, e]
    w_sb = const.tile([P, DC, E], F32)
    nc.sync.dma_start(out=w_sb[:], in_=w_ap[:])

    for rep in range(reps):
      for t in range(nt):
          x_sb = xpool.tile([P, D], F32)
          nc.sync.dma_start(out=x_sb[:], in_=x_ap[t * P:(t + 1) * P, :])

          # Transpose x tile to [d, tok] chunks via PE; 4 chunks per PSUM bank.
          xt_sb = xtpool.tile([P, D], F32)
          for h in range(2):
              ps = pst.tile([P, 4 * P], F32)
              for j in range(4):
                  c = 4 * h + j
                  nc.tensor.transpose(
                      ps[:, j * P:(j + 1) * P], x_sb[:, c * P:(c + 1) * P], ident[:]
                  )
              if h == 0:
                  nc.vector.tensor_copy(out=xt_sb[:, 0:4 * P], in_=ps[:])
              else:
                  nc.scalar.copy(out=xt_sb[:, 4 * P:8 * P], in_=ps[:])

          # logits[tok, e] accumulated over 8 d-chunks
          lg_ps = psl.tile([P, E], F32)
          for c in range(DC):
              nc.tensor.matmul(
                  lg_ps[:],
                  lhsT=xt_sb[:, c * P:(c + 1) * P],
                  rhs=w_sb[:, c, :],
                  start=(c == 0),
                  stop=(c == DC - 1),
              )
          lg = small.tile([P, E], F32)
          nc.vector.tensor_copy(out=lg[:], in_=lg_ps[:])

          # top-8 values (we use top-2) and their indices
          m8 = small.tile([P, 8], F32)
          i8 = small.tile([P, 8], mybir.dt.uint32)
          nc.vector.max(out=m8[:], in_=lg[:])
          nc.vector.max_index(out=i8[:], in_max=m8[:], in_values=lg[:])

          # softmax denominator Z = sum(exp(l - m1)); gates g1 = 1/Z, g2 = exp(m2-m1)/Z
          negm = small.tile([P, 1], F32)
          nc.vector.tensor_scalar_mul(out=negm[:], in0=m8[:, 0:1], scalar1=-1.0)
          esc = small.tile([P, E], F32)
          z = small.tile([P, 1], F32)
          nc.scalar.activation(
              out=esc[:], in_=lg[:], func=mybir.ActivationFunctionType.Exp,
              bias=negm[:], scale=1.0, accum_out=z[:],
          )
          g1 = small.tile([P, 1], F32)
          nc.vector.reciprocal(out=g1[:], in_=z[:])
          e2 = small.tile([P, 1], F32)
          nc.scalar.activation(
              out=e2[:], in_=m8[:, 1:2], func=mybir.ActivationFunctionType.Exp,
              bias=negm[:], scale=1.0,
          )
          g2 = small.tile([P, 1], F32)
          nc.vector.tensor_mul(out=g2[:], in0=e2[:], in1=g1[:])

          # gated dispatch: y[t] = [x*g1 | x*g2], one contiguous 1MB DMA out
          y_sb = ypool.tile([P, TOPK * D], F32)
          nc.vector.tensor_scalar_mul(out=y_sb[:, 0:D], in0=x_sb[:], scalar1=g1[:])
          nc.scalar.activation(
              out=y_sb[:, D:TOPK * D], in_=x_sb[:],
              func=mybir.ActivationFunctionType.Copy, scale=g2[:],
          )
          nc.gpsimd.dma_start(out=y_ap[t * P:(t + 1) * P, :], in_=y_sb[:])
          nc.sync.dma_start(
              out=idx_ap[t * P:(t + 1) * P, :],
              in_=i8[:, 0:TOPK].bitcast(mybir.dt.int32),
          )


def build(n_tiles=NT, reps=1):
    nc = bacc.Bacc(
        "TRN2", target_bir_lowering=False, debug=False, num_devices=N_CORES
    )
    tok = n_tiles * P
    x_ap = nc.dram_tensor("x", [tok, D], F32, kind="ExternalInput").ap()
    w_ap = nc.dram_tensor("w", [P, DC, E], F32, kind="ExternalInput").ap()
    y_ap = nc.dram_tensor("y", [tok, TOPK * D], F32, kind="ExternalOutput").ap()
    idx_ap = nc.dram_tensor("idx", [tok, TOPK], mybir.dt.int32, kind="ExternalOutput").ap()
    with tile.TileContext(nc) as tc:
        with ExitStack() as ctx:
            _moe_tile_kernel(ctx, tc, x_ap, w_ap, y_ap, idx_ap, nt=n_tiles, reps=reps)
    nc.compile()
    return nc


_NC_CACHE = {}


def run(inputs: np.ndarray, w: np.ndarray, trace: bool = False):
    """Run on 8 cores; returns ((out, idx), BassKernelResults)."""
    if "nc" not in _NC_CACHE:
        _NC_CACHE["nc"] = build()
    nc = _NC_CACHE["nc"]

    x = np.ascontiguousarray(np.asarray(inputs, dtype=np.float32).reshape(N_TOK, D))
    w = np.ascontiguousarray(np.asarray(w, dtype=np.float32))
    w_chunked = np.ascontiguousarray(w.reshape(DC, P, E).transpose(1, 0, 2))
    in_maps = [
        {"x": x[c * TOK_CORE:(c + 1) * TOK_CORE], "w": w_chunked}
        for c in range(N_CORES)
    ]
    res = run_bass_kernel_spmd(nc, in_maps, list(range(N_CORES)), trace=trace)
    y = np.concatenate([res.results[c]["y"] for c in range(N_CORES)], axis=0)
    idx = np.concatenate([res.results[c]["idx"] for c in range(N_CORES)], axis=0)
    out = y.reshape(N_TOK * TOPK, D)
    idx = idx.reshape(-1).astype(np.int32)
    return (out, idx), res


def kernel(inputs: np.ndarray, w: np.ndarray):
    (out, idx), _ = run(inputs, w)
    return out, idx



# revision 12
# speedup vs baseline: 1.7640x; 1.7640x over previous
"""MoE router + dispatch kernel for Trainium2 (Bass/Tile), 8-core data parallel.

Computes, for x = inputs.reshape(B*S, D):
    logits = x @ w                      # [N, E]
    probs  = softmax(logits, axis=-1)
    gate, idx = top_k(probs, k=2)       # [N, 2]
    out[2t+k] = x[t] * gate[t, k]       # [2N, D]
returns (out, idx.reshape(-1)) matching the jax reference.

Sharding: data-parallel on the token dim across 8 NeuronCores; w replicated.
"""

from contextlib import ExitStack

import numpy as np

import concourse.bacc as bacc
import concourse.bass as bass
import concourse.tile as tile
from concourse import mybir
from concourse.bass_utils import run_bass_kernel_spmd
from concourse.masks import make_identity

N_CORES = 8
B, S, D, E, TOPK = 8, 4096, 1024, 64, 2
N_TOK = B * S                  # 32768 tokens total
TOK_CORE = N_TOK // N_CORES    # 4096 tokens per core
P = 128                        # SBUF partitions
NT = TOK_CORE // P             # 32 token tiles per core
DC = D // P                    # 8 contraction chunks

F32 = mybir.dt.float32


def _moe_tile_kernel(ctx: ExitStack, tc: tile.TileContext, x_ap, w_ap, y_ap, idx_ap, nt=NT, reps=1):
    nc = tc.nc

    const = ctx.enter_context(tc.tile_pool(name="const", bufs=1))
    xpool = ctx.enter_context(tc.tile_pool(name="xp", bufs=10))
    xtpool = ctx.enter_context(tc.tile_pool(name="xtp", bufs=6))
    ypool = ctx.enter_context(tc.tile_pool(name="yp", bufs=6))
    small = ctx.enter_context(tc.tile_pool(name="small", bufs=12))
    pst = ctx.enter_context(tc.tile_pool(name="pst", bufs=4, space="PSUM"))
    psl = ctx.enter_context(tc.tile_pool(name="psl", bufs=4, space="PSUM"))

    ident = const.tile([P, P], F32)
    make_identity(nc, ident[:])

    # w pre-chunked on host: w_ap[p, c, e] = w[c*128 + p, e]
    w_sb = const.tile([P, DC, E], F32)
    nc.sync.dma_start(out=w_sb[:], in_=w_ap[:])
    idx_acc = const.tile([P, nt, TOPK], mybir.dt.int32)

    for rep in range(reps):
      for t in range(nt):
          x_sb = xpool.tile([P, D], F32)
          nc.sync.dma_start(out=x_sb[:], in_=x_ap[t * P:(t + 1) * P, :])

          # Transpose x tile to [d, tok] chunks via PE; 2 chunks per PSUM tile
          # so copies (alternating DVE/ACT) release banks sooner.
          xt_sb = xtpool.tile([P, D], F32)
          for h in range(4):
              ps = pst.tile([P, 2 * P], F32)
              for j in range(2):
                  c = 2 * h + j
                  nc.tensor.transpose(
                      ps[:, j * P:(j + 1) * P], x_sb[:, c * P:(c + 1) * P], ident[:]
                  )
              if h % 2 == 0:
                  nc.vector.tensor_copy(out=xt_sb[:, 2 * h * P:2 * (h + 1) * P], in_=ps[:])
              else:
                  nc.scalar.copy(out=xt_sb[:, 2 * h * P:2 * (h + 1) * P], in_=ps[:])

          # logits[tok, e] accumulated over 8 d-chunks
          lg_ps = psl.tile([P, E], F32)
          for c in range(DC):
              nc.tensor.matmul(
                  lg_ps[:],
                  lhsT=xt_sb[:, c * P:(c + 1) * P],
                  rhs=w_sb[:, c, :],
                  start=(c == 0),
                  stop=(c == DC - 1),
              )
          lg = small.tile([P, E], F32)
          nc.vector.tensor_copy(out=lg[:], in_=lg_ps[:])

          # top-8 values (we use top-2) and their indices
          m8 = small.tile([P, 8], F32)
          i8 = small.tile([P, 8], mybir.dt.uint32)
          nc.vector.max(out=m8[:], in_=lg[:])
          nc.vector.max_index(out=i8[:], in_max=m8[:], in_values=lg[:])
          nc.vector.tensor_copy(
              out=idx_acc[:, t, :], in_=i8[:, 0:TOPK].bitcast(mybir.dt.int32)
          )

          # softmax denominator Z = sum(exp(l - m1)); gates g1 = 1/Z, g2 = exp(m2-m1)/Z
          negm = small.tile([P, 1], F32)
          nc.vector.tensor_scalar_mul(out=negm[:], in0=m8[:, 0:1], scalar1=-1.0)
          esc = small.tile([P, E], F32)
          z = small.tile([P, 1], F32)
          nc.scalar.activation(
              out=esc[:], in_=lg[:], func=mybir.ActivationFunctionType.Exp,
              bias=negm[:], scale=1.0, accum_out=z[:],
          )
          g1 = small.tile([P, 1], F32)
          nc.vector.reciprocal(out=g1[:], in_=z[:])
          e2 = small.tile([P, 1], F32)
          nc.scalar.activation(
              out=e2[:], in_=m8[:, 1:2], func=mybir.ActivationFunctionType.Exp,
              bias=negm[:], scale=1.0,
          )
          g2 = small.tile([P, 1], F32)
          nc.vector.tensor_mul(out=g2[:], in0=e2[:], in1=g1[:])

          # gated dispatch: y[t] = [x*g1 | x*g2], one contiguous 1MB DMA out
          y_sb = ypool.tile([P, TOPK * D], F32)
          nc.vector.tensor_scalar_mul(out=y_sb[:, 0:D], in0=x_sb[:], scalar1=g1[:])
          nc.scalar.activation(
              out=y_sb[:, D:TOPK * D], in_=x_sb[:],
              func=mybir.ActivationFunctionType.Copy, scale=g2[:],
          )
          nc.gpsimd.dma_start(out=y_ap[t * P:(t + 1) * P, :], in_=y_sb[:])
      nc.sync.dma_start(
          out=idx_ap.rearrange("(t p) k -> p t k", p=P), in_=idx_acc[:]
      )


def build(n_tiles=NT, reps=1):
    nc = bacc.Bacc(
        "TRN2", target_bir_lowering=False, debug=False, num_devices=N_CORES
    )
    tok = n_tiles * P
    x_ap = nc.dram_tensor("x", [tok, D], F32, kind="ExternalInput").ap()
    w_ap = nc.dram_tensor("w", [P, DC, E], F32, kind="ExternalInput").ap()
    y_ap = nc.dram_tensor("y", [tok, TOPK * D], F32, kind="ExternalOutput").ap()
    idx_ap = nc.dram_tensor("idx", [tok, TOPK], mybir.dt.int32, kind="ExternalOutput").ap()
    with tile.TileContext(nc) as tc:
        with ExitStack() as ctx:
            _moe_tile_kernel(ctx, tc, x_ap, w_ap, y_ap, idx_ap, nt=n_tiles, reps=reps)
    nc.compile()
    return nc


_NC_CACHE = {}


def run(inputs: np.ndarray, w: np.ndarray, trace: bool = False):
    """Run on 8 cores; returns ((out, idx), BassKernelResults)."""
    if "nc" not in _NC_CACHE:
        _NC_CACHE["nc"] = build()
    nc = _NC_CACHE["nc"]

    x = np.ascontiguousarray(np.asarray(inputs, dtype=np.float32).reshape(N_TOK, D))
    w = np.ascontiguousarray(np.asarray(w, dtype=np.float32))
    w_chunked = np.ascontiguousarray(w.reshape(DC, P, E).transpose(1, 0, 2))
    in_maps = [
        {"x": x[c * TOK_CORE:(c + 1) * TOK_CORE], "w": w_chunked}
        for c in range(N_CORES)
    ]
    res = run_bass_kernel_spmd(nc, in_maps, list(range(N_CORES)), trace=trace)
    y = np.concatenate([res.results[c]["y"] for c in range(N_CORES)], axis=0)
    idx = np.concatenate([res.results[c]["idx"] for c in range(N_CORES)], axis=0)
    out = y.reshape(N_TOK * TOPK, D)
    idx = idx.reshape(-1).astype(np.int32)
    return (out, idx), res


def kernel(inputs: np.ndarray, w: np.ndarray):
    (out, idx), _ = run(inputs, w)
    return out, idx

